# revision 10
# baseline (speedup 1.0000x reference)
"""Trainium2 Bass kernel for nn_BaseCompressor2 (truncated one-pole IIR compressor).

Algorithm (per batch n, signal length L=262144, C=2 channels):
  energy[t] = mean_c(sig[c,t]^2)
  y = IIR(energy): y[t] = alpha*y[t-1] + (1-alpha)*energy[t]   (the reference
      truncates the impulse response at 16384 taps; alpha^2048 underflows to 0
      in fp32 for all realistic alpha=sigmoid(z), so the infinite IIR is
      bit-identical)
  x = ln(y + 1e-5); piecewise knee gain; out = exp(log_gain) * sig

Mapping: batch N=32 sharded 4-per-core across 8 cores (pure data parallel).
Per batch: both channels live in one [128, 2, 2048] tile; partition p = time
block [p*2048,(p+1)*2048). The IIR is a DVE tensor_tensor_scan along the free
dim (data0 = per-partition alpha via a stride-0 broadcast AP). The
cross-partition carry C[p] = y[p-1, -1] is formed by an idle-PE shift-matmul
into PSUM and applied as y[:, :T0] += alpha^(t+1)*C[p] with one
scalar_tensor_tensor over the first T0 columns only: beyond T0, alpha^(t+1)
underflows to exactly 0 in fp32, so the scan values are already exact. The
alpha^(t+1) table (pw) is built off the critical path on ACT from a one-time
iota. The final gain multiply covers both channels in one DVE op via a
stride-0 middle-dim broadcast of the gain tile.
"""

import numpy as np

N, C, L = 32, 2, 262144
NCORES = 8
BPC = N // NCORES  # batches per core
P = 128
FD = L // P  # 2048 free elems per partition

# pcols column layout (per batch b, base b*NP)
NP = 8
ALPHA, SQC, UK, KNEE, NEGC1, C1K2, SQC2, LNA = range(NP)

_cache = {}


def _host_params(z_alpha_pre, log_threshold, log_ratio, log_knee):
    """Per-batch derived scalars, float64 math -> float32 columns."""
    z = z_alpha_pre.astype(np.float64).reshape(-1)
    thr = log_threshold.astype(np.float64).reshape(-1) - 6.0
    knee = np.exp(log_knee.astype(np.float64).reshape(-1))
    r001 = 1.0 + np.exp(log_ratio.astype(np.float64).reshape(-1)) + 0.001
    alpha = 1.0 / (1.0 + np.exp(-z))
    # Carry-truncation validity: alpha^2048 must underflow to exactly 0 in f32.
    assert np.max(2048.0 * np.log(alpha)) < -88.0, "alpha too close to 1"
    c1 = 1.0 / r001 - 1.0  # < 0
    vals = np.zeros((N, NP), dtype=np.float64)
    vals[:, ALPHA] = alpha
    vals[:, SQC] = np.sqrt((1.0 - alpha) / 2.0)
    vals[:, UK] = knee / 2.0 - thr
    vals[:, KNEE] = knee
    vals[:, NEGC1] = -c1
    vals[:, C1K2] = c1 * knee / 2.0
    vals[:, SQC2] = np.sqrt(-c1 / (2.0 * (knee + 0.001)))
    vals[:, LNA] = np.log(alpha)
    # carry influence horizon: alpha^(t+1) == 0 in f32 for t >= T0
    t0 = int(np.ceil(88.0 / max(1e-9, -np.max(np.log(alpha))))) + 64
    t0 = min(FD, max(128, t0))
    return vals.astype(np.float32), t0


def _shift_matrix():
    # lhsT[k, m] = 1 iff m == k+1, so (lhsT.T @ f)[m] = f[m-1], row 0 -> 0
    m = np.zeros((P, P), dtype=np.float32)
    m[np.arange(P - 1), np.arange(1, P)] = 1.0
    return m


def _build_program(T0):
    from contextlib import ExitStack

    import concourse.bacc as bacc
    import concourse.bass as bass
    import concourse.tile as tile
    from concourse import mybir

    dt = mybir.dt.float32
    Alu = mybir.AluOpType
    Af = mybir.ActivationFunctionType

    nc = bacc.Bacc(
        "TRN2", target_bir_lowering=False, debug=False,
        enable_asserts=False, num_devices=NCORES,
    )
    sig = nc.dram_tensor("sig", [BPC, C, L], dt, kind="ExternalInput")
    pcols = nc.dram_tensor("pcols", [P, BPC * NP], dt, kind="ExternalInput")
    shiftm = nc.dram_tensor("shiftm", [P, P], dt, kind="ExternalInput")
    out = nc.dram_tensor("out", [BPC, C, L], dt, kind="ExternalOutput")

    def dram_pcx(tensor, b):
        # [P, C, FD] view of batch b: partition p, channel c, time-in-block t
        return bass.AP(tensor, b * C * L, [[FD, P], [L, C], [1, FD]])

    with tile.TileContext(nc) as tc, ExitStack() as ctx:
        const = ctx.enter_context(tc.tile_pool(name="const", bufs=1))
        io = ctx.enter_context(tc.tile_pool(name="io", bufs=3))
        wk = ctx.enter_context(tc.tile_pool(name="wk", bufs=2))
        psum = ctx.enter_context(tc.tile_pool(name="psum", bufs=2, space="PSUM"))

        pc = const.tile([P, BPC * NP], dt, tag="pc")
        nc.sync.dma_start(pc, pcols.ap())
        shift_sb = const.tile([P, P], dt, tag="shift")
        nc.sync.dma_start(shift_sb, shiftm.ap())
        zcol = const.tile([P, 1], dt, tag="zcol")
        nc.vector.memset(zcol, 0.0)
        epscol = const.tile([P, 1], dt, tag="epscol")
        nc.vector.memset(epscol, 1e-5)
        # one-time t+1 ramp for the alpha-power table
        iota_i = const.tile([P, T0], mybir.dt.int32, tag="iota_i")
        iota_f = const.tile([P, T0], dt, tag="iota_f")
        nc.gpsimd.iota(iota_i, pattern=[[1, T0]], base=1, channel_multiplier=0)
        nc.vector.tensor_copy(iota_f, iota_i)

        for b in range(BPC):
            def col(j, b=b):
                return pc[:, b * NP + j: b * NP + j + 1]

            s01 = io.tile([P, C, FD], dt, tag="s01")
            nc.sync.dma_start(s01, dram_pcx(sig, b))

            # pw[p, t] = alpha^(t+1), built off the critical path
            pw = wk.tile([P, T0], dt, tag="pw")
            nc.scalar.activation(pw, iota_f, Af.Exp, scale=col(LNA),
                                 bias=zcol[:, 0:1])

            # sq = (1-a)/2 * s01^2 (both channels, one ACT op)
            sq = wk.tile([P, C, FD], dt, tag="sq")
            nc.scalar.activation(sq, s01, Af.Square, scale=col(SQC),
                                 bias=zcol[:, 0:1])
            # b_t = sq[ch0] + sq[ch1] == (1-a)*energy, into sq[:, 0, :]
            nc.gpsimd.tensor_add(sq[:, 0, :], sq[:, 0, :], sq[:, 1, :])

            # scan: per-partition local IIR (initial 0) -> y
            y = wk.tile([P, FD], dt, tag="y")
            nc.vector.tensor_tensor_scan(y, col(ALPHA).to_broadcast((P, FD)),
                                         sq[:, 0, :], 0.0, Alu.mult, Alu.add)

            # carry C[p] = y[p-1, FD-1] via PE shift-matmul (C[0] = 0)
            c_ps = psum.tile([P, 1], dt, tag="C")
            nc.tensor.matmul(c_ps, shift_sb, y[:, FD - 1: FD],
                             start=True, stop=True)

            # apply carry on the first T0 columns: y += pw * C
            nc.vector.scalar_tensor_tensor(y[:, 0:T0], pw, c_ps[:, 0:1],
                                           y[:, 0:T0], Alu.mult, Alu.add)

            # x = ln(y + 1e-5); u = relu(x + knee/2 - thr)  (in place on y)
            nc.scalar.activation(y, y, Af.Ln, bias=epscol[:, 0:1])
            nc.scalar.activation(y, y, Af.Relu, bias=col(UK))

            # piecewise knee, h = -log_gain >= 0
            m_a = wk.tile([P, FD], mybir.dt.int8, tag="ma")
            ha = wk.tile([P, FD], dt, tag="ha")
            h = wk.tile([P, FD], dt, tag="h")
            nc.vector.tensor_scalar(m_a, y, col(KNEE), None, Alu.is_gt)
            nc.scalar.activation(ha, y, Af.Identity, scale=col(NEGC1),
                                 bias=col(C1K2))
            nc.scalar.activation(h, y, Af.Square, scale=col(SQC2),
                                 bias=zcol[:, 0:1])
            nc.vector.copy_predicated(h, m_a, ha)

            # gain = exp(-h) in place, then one multiply for both channels
            nc.scalar.activation(h, h, Af.Exp, scale=-1.0, bias=zcol[:, 0:1])
            h3 = bass.AP(h.tensor, h.offset, [h.ap[0], [0, C], h.ap[1]])
            nc.vector.tensor_tensor(s01, s01, h3, Alu.mult)
            nc.sync.dma_start(dram_pcx(out, b), s01)

    nc.compile()
    return nc


def _get_program(T0):
    key = ("nc", T0)
    if key not in _cache:
        _cache[key] = _build_program(T0)
    return _cache[key]


def _run(inputs, trace=False):
    from concourse.bass_utils import run_bass_kernel_spmd

    sig_full = np.ascontiguousarray(np.asarray(inputs["input_signals"], np.float32))
    pv, T0 = _host_params(
        np.asarray(inputs["z_alpha_pre"], np.float32),
        np.asarray(inputs["log_threshold"], np.float32),
        np.asarray(inputs["log_ratio"], np.float32),
        np.asarray(inputs["log_knee"], np.float32),
    )

    nc = _get_program(T0)
    shm = _shift_matrix()
    in_maps = []
    for k in range(NCORES):
        shard = np.ascontiguousarray(sig_full[k * BPC:(k + 1) * BPC])
        cols = np.broadcast_to(
            pv[k * BPC:(k + 1) * BPC].reshape(1, BPC * NP), (P, BPC * NP)
        )
        in_maps.append({"sig": shard, "pcols": np.ascontiguousarray(cols),
                        "shiftm": shm})

    res = run_bass_kernel_spmd(
        nc, in_maps, core_ids=list(range(NCORES)), trace=trace,
    )
    out = np.empty((N, C, L), dtype=np.float32)
    for k in range(NCORES):
        out[k * BPC:(k + 1) * BPC] = res.results[k]["out"]
    return out, res


def kernel(**inputs) -> np.ndarray:
    out, _ = _run(inputs, trace=False)
    return out


# revision 11
# speedup vs baseline: 1.0846x; 1.0846x over previous
"""Trainium2 Bass kernel for nn_BaseCompressor2 (truncated one-pole IIR compressor).

Algorithm (per batch n, signal length L=262144, C=2 channels):
  energy[t] = mean_c(sig[c,t]^2)
  y = IIR(energy): y[t] = alpha*y[t-1] + (1-alpha)*energy[t]   (the reference
      truncates the impulse response at 16384 taps; alpha^2048 underflows to 0
      in fp32 for all realistic alpha=sigmoid(z), so the infinite IIR is
      bit-identical)
  x = ln(y + 1e-5); piecewise knee gain; out = exp(log_gain) * sig

Mapping: batch N=32 sharded 4-per-core across 8 cores (pure data parallel).
Per batch: both channels live in one [128, 2, 2048] tile; partition p = time
block [p*2048,(p+1)*2048). The IIR is a DVE tensor_tensor_scan along the free
dim (data0 = per-partition alpha via a stride-0 broadcast AP). The
cross-partition carry C[p] = y[p-1, -1] is formed by an idle-PE shift-matmul
into PSUM and applied as y[:, :T0] += alpha^(t+1)*C[p] with one
scalar_tensor_tensor over the first T0 columns only: beyond T0, alpha^(t+1)
underflows to exactly 0 in fp32, so the scan values are already exact. The
alpha^(t+1) table (pw) is built off the critical path on ACT from a one-time
iota. The final gain multiply covers both channels in one DVE op via a
stride-0 middle-dim broadcast of the gain tile.
"""

import numpy as np

N, C, L = 32, 2, 262144
NCORES = 8
BPC = N // NCORES  # batches per core
P = 128
FD = L // P  # 2048 free elems per partition

# pcols column layout (per batch b, base b*NP)
NP = 8
ALPHA, SQC, UK, KNEE, NEGC1, C1K2, SQC2, LNA = range(NP)

_cache = {}


def _host_params(z_alpha_pre, log_threshold, log_ratio, log_knee):
    """Per-batch derived scalars, float64 math -> float32 columns."""
    z = z_alpha_pre.astype(np.float64).reshape(-1)
    thr = log_threshold.astype(np.float64).reshape(-1) - 6.0
    knee = np.exp(log_knee.astype(np.float64).reshape(-1))
    r001 = 1.0 + np.exp(log_ratio.astype(np.float64).reshape(-1)) + 0.001
    alpha = 1.0 / (1.0 + np.exp(-z))
    # Carry-truncation validity: alpha^2048 must underflow to exactly 0 in f32.
    assert np.max(2048.0 * np.log(alpha)) < -88.0, "alpha too close to 1"
    c1 = 1.0 / r001 - 1.0  # < 0
    vals = np.zeros((N, NP), dtype=np.float64)
    vals[:, ALPHA] = alpha
    vals[:, SQC] = np.sqrt((1.0 - alpha) / 2.0)
    vals[:, UK] = knee / 2.0 - thr
    vals[:, KNEE] = knee
    vals[:, NEGC1] = -c1
    vals[:, C1K2] = c1 * knee / 2.0
    vals[:, SQC2] = np.sqrt(-c1 / (2.0 * (knee + 0.001)))
    vals[:, LNA] = np.log(alpha)
    # carry influence horizon: alpha^(t+1) == 0 in f32 for t >= T0
    t0 = int(np.ceil(88.0 / max(1e-9, -np.max(np.log(alpha))))) + 64
    t0 = min(FD, max(128, t0))
    return vals.astype(np.float32), t0


def _shift_matrix():
    # lhsT[k, m] = 1 iff m == k+1, so (lhsT.T @ f)[m] = f[m-1], row 0 -> 0
    m = np.zeros((P, P), dtype=np.float32)
    m[np.arange(P - 1), np.arange(1, P)] = 1.0
    return m


def _build_program(T0):
    from contextlib import ExitStack

    import concourse.bacc as bacc
    import concourse.bass as bass
    import concourse.tile as tile
    from concourse import mybir

    dt = mybir.dt.float32
    Alu = mybir.AluOpType
    Af = mybir.ActivationFunctionType

    nc = bacc.Bacc(
        "TRN2", target_bir_lowering=False, debug=False,
        enable_asserts=False, num_devices=NCORES,
    )
    sig = nc.dram_tensor("sig", [BPC, C, L], dt, kind="ExternalInput")
    pcols = nc.dram_tensor("pcols", [P, BPC * NP], dt, kind="ExternalInput")
    shiftm = nc.dram_tensor("shiftm", [P, P], dt, kind="ExternalInput")
    pwt = nc.dram_tensor("pwt", [BPC, T0], dt, kind="ExternalInput")
    out = nc.dram_tensor("out", [BPC, C, L], dt, kind="ExternalOutput")

    def dram_pcx(tensor, b):
        # [P, C, FD] view of batch b: partition p, channel c, time-in-block t
        return bass.AP(tensor, b * C * L, [[FD, P], [L, C], [1, FD]])

    with tile.TileContext(nc) as tc, ExitStack() as ctx:
        const = ctx.enter_context(tc.tile_pool(name="const", bufs=1))
        io = ctx.enter_context(tc.tile_pool(name="io", bufs=3))
        wk = ctx.enter_context(tc.tile_pool(name="wk", bufs=2))
        psum = ctx.enter_context(tc.tile_pool(name="psum", bufs=2, space="PSUM"))

        pc = const.tile([P, BPC * NP], dt, tag="pc")
        nc.sync.dma_start(pc, pcols.ap())
        shift_sb = const.tile([P, P], dt, tag="shift")
        nc.sync.dma_start(shift_sb, shiftm.ap())
        zcol = const.tile([P, 1], dt, tag="zcol")
        nc.vector.memset(zcol, 0.0)
        epscol = const.tile([P, 1], dt, tag="epscol")
        nc.vector.memset(epscol, 1e-5)

        for b in range(BPC):
            def col(j, b=b):
                return pc[:, b * NP + j: b * NP + j + 1]

            s01 = io.tile([P, C, FD], dt, tag="s01")
            nc.sync.dma_start(s01, dram_pcx(sig, b))

            # pw[p, t] = alpha^(t+1): host-precomputed, partition-broadcast DMA
            pw = wk.tile([P, T0], dt, tag="pw")
            nc.sync.dma_start(pw, bass.AP(pwt, b * T0, [[0, P], [1, T0]]))

            # sq = (1-a)/2 * s01^2 (both channels, one ACT op)
            sq = wk.tile([P, C, FD], dt, tag="sq")
            nc.scalar.activation(sq, s01, Af.Square, scale=col(SQC),
                                 bias=zcol[:, 0:1])
            # b_t = sq[ch0] + sq[ch1] == (1-a)*energy, into sq[:, 0, :]
            nc.vector.tensor_add(sq[:, 0, :], sq[:, 0, :], sq[:, 1, :])

            # scan: per-partition local IIR (initial 0) -> y
            y = wk.tile([P, FD], dt, tag="y")
            nc.vector.tensor_tensor_scan(y, col(ALPHA).to_broadcast((P, FD)),
                                         sq[:, 0, :], 0.0, Alu.mult, Alu.add)

            # carry C[p] = y[p-1, FD-1] via PE shift-matmul (C[0] = 0)
            c_ps = psum.tile([P, 1], dt, tag="C")
            nc.tensor.matmul(c_ps, shift_sb, y[:, FD - 1: FD],
                             start=True, stop=True)

            # apply carry on the first T0 columns: y += pw * C
            nc.vector.scalar_tensor_tensor(y[:, 0:T0], pw, c_ps[:, 0:1],
                                           y[:, 0:T0], Alu.mult, Alu.add)

            # x = ln(y + 1e-5); u = relu(x + knee/2 - thr)  (in place on y)
            nc.scalar.activation(y, y, Af.Ln, bias=epscol[:, 0:1])
            nc.scalar.activation(y, y, Af.Relu, bias=col(UK))

            # piecewise knee, h = -log_gain >= 0
            m_a = wk.tile([P, FD], mybir.dt.int8, tag="ma")
            ha = wk.tile([P, FD], dt, tag="ha")
            h = wk.tile([P, FD], dt, tag="h")
            nc.vector.tensor_scalar(m_a, y, col(KNEE), None, Alu.is_gt)
            nc.scalar.activation(ha, y, Af.Identity, scale=col(NEGC1),
                                 bias=col(C1K2))
            nc.scalar.activation(h, y, Af.Square, scale=col(SQC2),
                                 bias=zcol[:, 0:1])
            nc.vector.copy_predicated(h, m_a, ha)

            # gain = exp(-h) in place, then one multiply for both channels
            nc.scalar.activation(h, h, Af.Exp, scale=-1.0, bias=zcol[:, 0:1])
            h3 = bass.AP(h.tensor, h.offset, [h.ap[0], [0, C], h.ap[1]])
            nc.vector.tensor_tensor(s01, s01, h3, Alu.mult)
            nc.sync.dma_start(dram_pcx(out, b), s01)

    nc.compile()
    return nc


def _get_program(T0):
    key = ("nc", T0)
    if key not in _cache:
        _cache[key] = _build_program(T0)
    return _cache[key]


def _run(inputs, trace=False):
    from concourse.bass_utils import run_bass_kernel_spmd

    sig_full = np.ascontiguousarray(np.asarray(inputs["input_signals"], np.float32))
    pv, T0 = _host_params(
        np.asarray(inputs["z_alpha_pre"], np.float32),
        np.asarray(inputs["log_threshold"], np.float32),
        np.asarray(inputs["log_ratio"], np.float32),
        np.asarray(inputs["log_knee"], np.float32),
    )

    nc = _get_program(T0)
    shm = _shift_matrix()
    zf = np.asarray(inputs["z_alpha_pre"], np.float64).reshape(-1)
    alpha64 = 1.0 / (1.0 + np.exp(-zf))
    tpow = np.arange(1, T0 + 1, dtype=np.float64)
    pw_all = np.exp(tpow[None, :] * np.log(alpha64)[:, None]).astype(np.float32)
    in_maps = []
    for k in range(NCORES):
        shard = np.ascontiguousarray(sig_full[k * BPC:(k + 1) * BPC])
        cols = np.broadcast_to(
            pv[k * BPC:(k + 1) * BPC].reshape(1, BPC * NP), (P, BPC * NP)
        )
        in_maps.append({"sig": shard, "pcols": np.ascontiguousarray(cols),
                        "shiftm": shm,
                        "pwt": np.ascontiguousarray(pw_all[k * BPC:(k + 1) * BPC])})

    res = run_bass_kernel_spmd(
        nc, in_maps, core_ids=list(range(NCORES)), trace=trace,
    )
    out = np.empty((N, C, L), dtype=np.float32)
    for k in range(NCORES):
        out[k * BPC:(k + 1) * BPC] = res.results[k]["out"]
    return out, res


def kernel(**inputs) -> np.ndarray:
    out, _ = _run(inputs, trace=False)
    return out
